# revision 26
# baseline (speedup 1.0000x reference)
"""Trainium2 Bass kernel for nn_LocalizeAttention (27-point 3D neighbourhood gather).

out[b,h,(pi,pj,pk),(di,dj,dk),d] = x[b,h,(pi+di-1, pj+dj-1, pk+dk-1),d], zero outside.

Strategy (per core, SPMD over 8 cores; 2 (b,h) volumes per core):
  - host zero-pads each volume to [26,26,26,32]
  - partition rows = (v 2, pi 24, pjo 8) = 384 = 3 exact 128-partition tiles
    (pji=3 keeps all 16 SDMA engines evenly loaded and cuts per-partition
    copy work 25% vs a 96-partition layout)
  - per partition-tile, 3 slabs (one per di'); slab free dim = (pj 3-wide + 2
    halo, pk_padded 26, d 32) so dj'/dk' are free-dim offsets. Ptile 0 loads
    all 3 slabs from HBM; ptiles 1-2 load only the di'=0 slab + 2 extra rows
    and synthesize the di'=1,2 partition-shifted slabs on the tensor engine
    (matmul with 0/1 shift matrices, accumulated from slab+extra, PSUM
    drained by vector/scalar) - this removes 2/3 of the HBM read traffic
  - 9 shifted copies per output tile assemble [128, (pjl 3, pkl pkb, s 27, d 32)];
    the 3 dk' merge into one contiguous 96-float run; copies are split across
    Vector/Scalar/GpSimd by greedy list scheduling
  - slabs double-buffered across partition-tiles; 4 rotating output buffers;
    all DMAs on the sync HWDGE ring, stores emitted in predicted-finish order;
    load descriptors split so each DMA spans all 16 SDMA engines
"""

import numpy as np

B, HEADS, DH = 2, 8, 32
H = W = D = 24
N = H * W * D
FN = 27
NCORES = 8
NVOL = (B * HEADS) // NCORES  # 2 volumes per core
USE_PE = True                 # tensor-engine shift for ptiles 1-2

# copy-engine speed estimates (ns per f32 element per partition), refined
# from trace measurements; used only for load balancing / emission order
RATES = {"V": 1.10, "A": 1.13, "G": 3.15}


def _ptile_segments():
    """3 partition-tiles of 128 rows; rows = (v, pi, pjo) with pjo in [0,8).
    Returns per tile a list of (v, pi0, npi, p0, pstep) segments: rows of the
    segment occupy partitions p0, p0+pstep, ... For the volume-spanning tile
    the two segments are interleaved (pstep=2) so every DMA covers all 16
    SDMA engines instead of starving half of them."""
    return [
        [(0, 0, 16, 0, 1)],
        [(0, 16, 8, 0, 2), (1, 0, 8, 1, 2)],
        [(1, 8, 16, 0, 1)],
    ]


def _layout_of(T):
    # 0 = stride-1 single-volume tile, 1 = volume-interleaved tile
    return 1 if T == 1 else 0


def build_w():
    """Shift matrices for the PE path: slab_di = W1.T @ slab0 + W2.T @ extra.
    wsh[idx, k, m] with idx = layout*4 + (di-1)*2 + which."""
    w = np.zeros((8, 128, 128), dtype=np.float32)
    for L in (0, 1):
        for di in (1, 2):
            for p in range(128):
                if L == 0:
                    k = p + 8 * di
                    if k < 128:
                        w[L * 4 + (di - 1) * 2 + 0, k, p] = 1.0
                    else:
                        w[L * 4 + (di - 1) * 2 + 1, k - 128, p] = 1.0
                else:
                    k = p + 16 * di
                    if k < 128:
                        w[L * 4 + (di - 1) * 2 + 0, k, p] = 1.0
                    else:
                        v, kk = p & 1, p >> 1
                        rr = kk + 8 * di - 64
                        w[L * 4 + (di - 1) * 2 + 1, v * 16 + rr, p] = 1.0
    return w


def extra_rows(T):
    """Rows loaded into the 'extra' tensor for ptile T: list of
    (v, pi_pad_row0, pbase) - 2 consecutive padded pi rows x 8 pjo each."""
    if T == 1:
        return [(0, 24, 0), (1, 8, 16)]
    if T == 2:
        return [(1, 24, 0)]
    return []


def _build_nc(nvol, pkb=2, nobuf=4, rates=RATES):
    import concourse.bass as bass
    import concourse.mybir as mybir
    from concourse.ap import AP
    from concourse.bacc import Bacc
    from concourse.tile import TileContext

    f32 = mybir.dt.float32
    dh = DH
    hp = wp = dp = 26
    pji, pjo = 3, 8
    P = 128
    fn = FN
    s_jp = dp * dh                 # 832: xpad pj stride (elements)
    s_ip = wp * s_jp               # 21632: xpad pi stride
    vol_pad = hp * s_ip
    slab_f = (pji + 2) * s_jp      # 4160
    out_f = pji * pkb * fn * dh    # otile free size
    run = 3 * dh                   # merged (dk', d) contiguous run
    ntile_k = D // pkb             # pk blocks per partition-tile
    vol_out = N * fn * dh
    row_out = pji * D * fn * dh    # 62208: output elems per partition row
    pjl_out = D * fn * dh          # 20736: output elems per pj line
    segs = _ptile_segments()
    NT = len(segs)

    nc = Bacc()
    xpad = nc.declare_dram_parameter("xpad", [nvol, hp, wp, dp, dh], f32,
                                     isOutput=False)
    if USE_PE:
        # k-major layout: one contiguous 4KB run per partition row
        wsh = nc.declare_dram_parameter("wsh", [128, 8, 128], f32,
                                        isOutput=False)
    out = nc.declare_dram_parameter("out", [nvol, N, fn, dh], f32,
                                    isOutput=True)
    xt = xpad[:].tensor
    ot = out[:].tensor

    import contextlib
    with contextlib.ExitStack() as ctx:
        tc = ctx.enter_context(TileContext(nc))
        # slabs[set][di]; 2 sets double-buffer across partition-tiles
        slabs = [[ctx.enter_context(
            nc.sbuf_tensor(f"slab{s}_{i}", [P, slab_f], f32))
            for i in range(3)] for s in range(2)]
        otiles = [ctx.enter_context(nc.sbuf_tensor(f"otile{i}", [P, out_f], f32))
                  for i in range(nobuf)]
        scratch = ctx.enter_context(nc.sbuf_tensor("scratch", [P, 32], f32))
        if USE_PE:
            extra = ctx.enter_context(nc.sbuf_tensor("extra", [P, slab_f], f32))
            wsb = ctx.enter_context(nc.sbuf_tensor("wsb", [P, 8 * 128], f32))
            psums = [ctx.enter_context(nc.psum_tensor(f"ps{i}", [P, 512], f32))
                     for i in range(4)]

        dense_f = pji * s_jp           # 2496: non-overlapping window part
        halo_f = slab_f - dense_f      # 1664: 2-row halo (re-read)

        def emit_loads(T, di):
            # split each slab load into a dense part (descriptors sweep DRAM
            # sequentially at full rate) and the overlapping halo part.
            # Dense descriptors are halved (2 per row) so each load spans
            # enough descriptor blocks to cover all 16 SDMA engines.
            slab = slabs[T % 2][di][:]
            for (v, pi0, npi, p0, pstep) in segs[T]:
                base = v * vol_pad + (pi0 + di) * s_ip
                src = AP(xt, base,
                         [[s_ip, npi], [dense_f // 2, 2 * pjo],
                          [1, dense_f // 2]])
                dst = AP(slab.tensor, slab.offset + p0 * slab_f,
                         [[pstep * slab_f, npi * pjo], [dense_f // 2, 2],
                          [1, dense_f // 2]])
                nc.sync.dma_start(out=dst, in_=src)
                src = AP(xt, base + dense_f,
                         [[s_ip, npi], [dense_f, pjo], [1, halo_f]])
                dst = AP(slab.tensor, slab.offset + p0 * slab_f + dense_f,
                         [[pstep * slab_f, npi * pjo], [1, halo_f]])
                nc.sync.dma_start(out=dst, in_=src)

        def emit_extra_loads(T):
            ex = extra[:]
            for (v, row0, pbase) in extra_rows(T):
                base = v * vol_pad + row0 * s_ip
                src = AP(xt, base,
                         [[s_ip, 2], [dense_f, pjo], [1, slab_f]])
                dst = AP(ex.tensor, ex.offset + pbase * slab_f,
                         [[slab_f, 16], [1, slab_f]])
                nc.sync.dma_start(out=dst, in_=src)

        engs = {"V": nc.vector, "A": nc.scalar, "G": nc.gpsimd}

        def copy(ename, dst_ap, src_ap):
            e = engs[ename]
            if hasattr(e, "tensor_copy"):
                e.tensor_copy(out=dst_ap, in_=src_ap)
            else:
                e.copy(out=dst_ap, in_=src_ap)

        def emit_shifts(T, part, nparts=3):
            # synthesize slabs[T%2][1], [2] = partition-shifted slab0 on PE;
            # emitted in `nparts` bursts so V/A never stall long behind PE
            L = _layout_of(T)
            slab0 = slabs[T % 2][0][:]
            ex = extra[:]
            work = []
            for di in (1, 2):
                for c0 in range(0, slab_f, 512):
                    work.append((di, c0))
            ci0 = part * len(work) // nparts
            ci1 = (part + 1) * len(work) // nparts
            for ci in range(ci0, ci1):
                di, c0 = work[ci]
                dst_t = slabs[T % 2][di][:]
                w1 = AP(wsb[:].tensor,
                        wsb[:].offset + (L * 4 + (di - 1) * 2) * 128,
                        [[8 * 128, 128], [1, 128]])
                w2 = AP(wsb[:].tensor,
                        wsb[:].offset + (L * 4 + (di - 1) * 2 + 1) * 128,
                        [[8 * 128, 32], [1, 128]])
                cw = min(512, slab_f - c0)
                ps = psums[ci % 4][:]
                pap = AP(ps.tensor, ps.offset, [[512, 128], [1, cw]])
                nc.tensor.matmul(
                    pap, w1,
                    AP(slab0.tensor, slab0.offset + c0,
                       [[slab_f, 128], [1, cw]]),
                    start=True, stop=False)
                nc.tensor.matmul(
                    pap, w2,
                    AP(ex.tensor, ex.offset + c0,
                       [[slab_f, 32], [1, cw]]),
                    start=False, stop=True)
                copy("V" if ci % 2 else "A",
                     AP(dst_t.tensor, dst_t.offset + c0,
                        [[slab_f, 128], [1, cw]]),
                     pap)

        # greedy schedule: per partition-tile, assign pk-blocks to engines by
        # earliest predicted finish; emit in predicted-finish order
        clock = {k: 0.0 for k in rates}
        tile_ns = {k: rates[k] * pji * pkb * run * 3 * 3 for k in rates}
        tix = 0
        if USE_PE:
            # zero 'extra' once (unused partitions must not hold NaNs: they
            # feed the W2 matmul, and NaN * 0.0 = NaN) and stage the W mats
            nc.vector.memzero(extra[:])
            wt = wsh[:].tensor
            wdst = AP(wsb[:].tensor, wsb[:].offset,
                      [[8 * 128, 128], [1, 8 * 128]])
            wsrc = AP(wt, 0, [[8 * 128, 128], [1, 8 * 128]])
            nc.sync.dma_start(out=wdst, in_=wsrc)
        for di in range(3):
            emit_loads(0, di)
        for T in range(NT):
            # wait-absorbers: soak the slab-DMA waits on each copy engine
            # (read one element from both the dense and halo regions)
            for ei, ename in enumerate(engs):
                for di in range(3):
                    slab = slabs[T % 2][di][:]
                    col = (ei * 3 + di) * 2
                    copy(ename,
                         AP(scratch[:].tensor, scratch[:].offset + col,
                            [[32, P], [1, 2]]),
                         AP(slab.tensor, slab.offset,
                            [[slab_f, P], [dense_f, 2]]))
            sched = []
            for t in range(ntile_k):
                ename = min(rates, key=lambda k: clock[k] + tile_ns[k])
                clock[ename] += tile_ns[ename]
                sched.append((clock[ename], ename, t))
            sched.sort()
            for k, (_fin, ename, t) in enumerate(sched):
                if T + 1 < NT:
                    if USE_PE:
                        if k == 2:
                            emit_loads(T + 1, 0)
                            emit_extra_loads(T + 1)
                        elif k in (4, 6, 8):
                            emit_shifts(T + 1, (k - 4) // 2)
                    elif k in (2, 5, 8):
                        emit_loads(T + 1, k // 3)
                otile = otiles[tix % nobuf][:]
                tix += 1
                for di in range(3):
                    slab = slabs[T % 2][di][:]
                    for dj in range(3):
                        src = AP(slab.tensor,
                                 slab.offset + dj * s_jp + t * pkb * dh,
                                 [[slab_f, P], [s_jp, pji], [dh, pkb],
                                  [1, run]])
                        dst = AP(otile.tensor,
                                 otile.offset + (di * 9 + dj * 3) * dh,
                                 [[out_f, P], [pkb * fn * dh, pji],
                                  [fn * dh, pkb], [1, run]])
                        copy(ename, dst, src)
                for (v, pi0, npi, p0, pstep) in segs[T]:
                    nrows = npi * pjo
                    sdst = AP(ot, v * vol_out + pi0 * pjo * row_out
                              + t * pkb * fn * dh,
                              [[row_out, nrows], [pjl_out, pji],
                               [1, pkb * fn * dh]])
                    ssrc = AP(otile.tensor, otile.offset + p0 * out_f,
                              [[pstep * out_f, nrows], [1, out_f]])
                    nc.sync.dma_start(out=sdst, in_=ssrc)

    nc.finalize()
    return nc


def _pad_volumes(x):
    # x: [nvol, N, dh] -> [nvol, hp, wp, dp, dh] zero-padded
    nvol = x.shape[0]
    xv = x.reshape(nvol, H, W, D, DH)
    xp = np.zeros((nvol, H + 2, W + 2, D + 2, DH), dtype=np.float32)
    xp[:, 1:H + 1, 1:W + 1, 1:D + 1, :] = xv
    return xp


def _run(x, trace=False):
    from concourse.bass_utils import run_bass_kernel_spmd

    x = np.asarray(x, dtype=np.float32)
    assert x.shape == (B, HEADS, N, DH), x.shape
    xf = x.reshape(B * HEADS, N, DH)
    nc = _build_nc(NVOL)
    w = None
    if USE_PE:
        # transpose to k-major [128, 8, 128] to match the DRAM declaration
        w = np.ascontiguousarray(build_w().transpose(1, 0, 2))
    in_maps = []
    for i in range(NCORES):
        m = {"xpad": _pad_volumes(xf[i * NVOL:(i + 1) * NVOL])}
        if USE_PE:
            m["wsh"] = w
        in_maps.append(m)
    res = run_bass_kernel_spmd(nc, in_maps, list(range(NCORES)), trace=trace)
    outs = np.concatenate([res.results[i]["out"] for i in range(NCORES)],
                          axis=0)
    return outs.reshape(B, HEADS, N, FN, DH), res


def kernel(x, height, width, depth, **_):
    assert int(height) == H and int(width) == W and int(depth) == D
    out, _res = _run(x, trace=False)
    return out


def kernel_profiled(x):
    out, res = _run(x, trace=True)
    return out, res
